# revision 29
# baseline (speedup 1.0000x reference)
"""GAT layer on 8 Trainium2 NeuronCores (Bass/Tile).

Strategy (dense slot packing + PE segment-reduce):
  - Targets sharded across 8 cores (12500 each), identity order. Block b
    = target ranks [128b, 128b+128). Sources split in 4 chunks so table
    indices fit int16.
  - Phase B builds a DRAM table row per node: [p bf16(128) | alpha_src
    f32(8) | junk] = 512B rows via PE matmuls (lhsT = xT tile bf16,
    rhs = pack0 = [W_proj | W_proj @ Ablk] bf16).
  - Phase C: edges packed DENSELY into slots per (block, chunk):
    slot k (slice s=k//128, partition p=k%128) holds edge k (src-sorted)
    of that (b,ch); W[b,ch] = ceil(max-over-cores count/128) slices.
    Padded slots have trank=255 and point at spread real rows.
  - dma_gather per (group-of-4-blocks, chunk) pulls 512B rows.
  - Per-slot target rank (trank) drives on-chip one-hot matrices:
      OH_jt[j,t] = (trank[j]==t) via DVE is_equal vs iota constants,
      OH_tj via PE broadcast of trankF + compare vs partition iota.
    beta per slot = PE matmul (lhsT=OH_tj slice, rhs=beta_block).
    U/D per target = PE accumulation over slices
      (lhsT=OH_jt slice, rhs=[E*p | E]).
  - s = alpha + beta, E = exp(0.2 s) * exp(0.8 relu(s)) (lrelu slope
    0.2), out = U/(D+eps) + skip, ELU.
"""

import os
import sys

sys.path.insert(0, "/opt/trn_rl_repo")

import numpy as np
from contextlib import ExitStack


def _ensure_ntff_hook():
    """Register the NTFF profile hook if the image's antenv lacks it."""
    try:
        import antenv.axon_hooks  # noqa: F401
        return
    except ImportError:
        pass
    try:
        import types
        import antenv
        from trn_agent_boot.trn_boot import _ntff_profile_via_ctypes
        hook = _ntff_profile_via_ctypes("/opt/axon/libaxon_pjrt.so")
        mod = types.ModuleType("antenv.axon_hooks")
        state = {"hook": hook}
        mod.get_axon_ntff_profile_hook = lambda: state["hook"]
        mod.set_axon_ntff_profile_hook = lambda h: state.update(hook=h)
        sys.modules["antenv.axon_hooks"] = mod
        antenv.axon_hooks = mod
    except Exception:
        pass


_ensure_ntff_hook()

import concourse.bass as bass
import concourse.bacc as bacc
import concourse.tile as tile
from concourse import mybir
from concourse._compat import cdiv
from concourse.bass_utils import run_bass_kernel_spmd

N_NODES = 100000
N_EDGES = 1600000
IN_F = 128
H = 8
F = 16
HF = H * F  # 128
NEG_SLOPE = 0.2
EPS = 1e-16
N_CORES = 8
TGT_PER_CORE = N_NODES // N_CORES  # 12500
N_BLOCKS = cdiv(TGT_PER_CORE, 128)  # 98
TGT_PAD = N_BLOCKS * 128  # 12544
CHUNK = 196 * 128  # 25088 nodes per chunk (int16-addressable)
N_CHUNKS = 4
CHUNK_NODES = [CHUNK, CHUNK, CHUNK, N_NODES - 3 * CHUNK]  # last: 24736
CHUNK_ROWS = [n + 1 for n in CHUNK_NODES]
CHUNK_BASE = [0]
for _c in range(1, N_CHUNKS):
    CHUNK_BASE.append(CHUNK_BASE[-1] + CHUNK_ROWS[_c - 1])
TABLE_ROWS = CHUNK_BASE[-1] + CHUNK_ROWS[-1]  # 100004
ROW_ELEMS = 256  # bf16 elems per table row (512B)
GRP = 2  # blocks per gather group
N_GRPS = cdiv(N_BLOCKS, GRP)  # 49

_COMPILED = {}


def _host_prep(x, edge_index, W_proj, W_skip, a_src, a_tgt):
    """Index/layout prep. Returns (common, per_core list)."""
    x = np.asarray(x, np.float32)
    ei = np.asarray(edge_index)
    src = ei[0].astype(np.int64)
    tgt = ei[1].astype(np.int64)

    import ml_dtypes
    bf16 = ml_dtypes.bfloat16
    xT16 = np.ascontiguousarray(x.T).astype(bf16)

    # pack0 = [W_proj | W_proj @ Ablk], pack2 = [W_skip | W_proj @ Bblk]
    Wp = np.asarray(W_proj, np.float32)
    Ws = np.asarray(W_skip, np.float32)
    Ablk = np.zeros((HF, H), np.float32)
    Bblk = np.zeros((HF, H), np.float32)
    asr = np.asarray(a_src, np.float32).reshape(H, F)
    atg = np.asarray(a_tgt, np.float32).reshape(H, F)
    for h in range(H):
        Ablk[h * F:(h + 1) * F, h] = asr[h]
        Bblk[h * F:(h + 1) * F, h] = atg[h]
    pack0 = np.concatenate([Wp, Wp @ Ablk], axis=1).astype(bf16)  # [128,136]
    pack2 = np.concatenate([Ws, Wp @ Bblk], axis=1).astype(bf16)

    iotaF = np.tile(np.arange(128, dtype=np.float32), (128, 1)).astype(bf16)
    iotaP = np.arange(128, dtype=np.float32).reshape(128, 1).astype(bf16)
    ones1 = np.ones((1, 128), np.float32).astype(bf16)

    chunk_of = np.minimum(src // CHUNK, N_CHUNKS - 1)
    local_of = (src - chunk_of * CHUNK).astype(np.int64)

    # per-core per-(block,chunk) edge lists (src-sorted)
    cores = []
    cnt_all = np.zeros((N_CORES, N_BLOCKS, N_CHUNKS), np.int64)
    for c in range(N_CORES):
        lo, hi = c * TGT_PER_CORE, (c + 1) * TGT_PER_CORE
        m = (tgt >= lo) & (tgt < hi)
        s_loc = local_of[m]
        s_ch = chunk_of[m]
        t_loc = (tgt[m] - lo).astype(np.int64)
        blk = t_loc // 128
        rnk = t_loc % 128
        # sort edges by (block, chunk, src)
        eo = np.lexsort((s_loc, s_ch, blk))
        cores.append(dict(s_loc=s_loc[eo], s_ch=s_ch[eo], blk=blk[eo],
                          rnk=rnk[eo]))
        np.add.at(cnt_all[c], (blk[eo], s_ch[eo]), 1)

    W = np.maximum(cdiv_arr(cnt_all.max(axis=0), 128), 0).astype(np.int64)
    d_tot = W.sum(axis=1)  # [98]
    D_total = int(d_tot.sum())
    C_total = int(8 * W.sum())

    # column offsets
    idx_off = np.zeros((N_BLOCKS, N_CHUNKS), np.int64)  # in slices
    pos = 0
    # gather order: per group, per chunk, per block-in-group
    goff = {}
    for g in range(N_GRPS):
        bs = range(g * GRP, min((g + 1) * GRP, N_BLOCKS))
        for ch in range(N_CHUNKS):
            for b in bs:
                goff[(b, ch)] = pos
                pos += int(W[b, ch])
    S_total = pos  # total slices

    per_core = []
    for c in range(N_CORES):
        d = cores[c]
        # per (b, ch) fill slots
        idx_slices = np.zeros((S_total, 128), np.int64)  # [slice, p] idx vals
        trank_slices = np.full((S_total, 128), 255, np.int64)
        # group edges
        key = d["blk"] * N_CHUNKS + d["s_ch"]
        uk, starts = np.unique(key, return_index=True)
        ends = np.append(starts[1:], len(key))
        for u, st, en in zip(uk, starts, ends):
            b, ch = int(u) // N_CHUNKS, int(u) % N_CHUNKS
            n = en - st
            w = int(W[b, ch])
            base = goff[(b, ch)]
            k = np.arange(128 * w)
            vals = np.empty(128 * w, np.int64)
            vals[:n] = d["s_loc"][st:en]
            npad = 128 * w - n
            if npad:
                vals[n:] = (np.arange(npad) * 97) % CHUNK_NODES[ch]
            idx_slices[base:base + w] = vals.reshape(w, 128)
            tr = np.full(128 * w, 255, np.int64)
            tr[:n] = d["rnk"][st:en]
            trank_slices[base:base + w] = tr.reshape(w, 128)
        # idxs tensor: per slice-run of each (grp,ch) the flat j order is
        # slice-major; wrap to [16, ...] and tile to [128, ...]
        flat = idx_slices.reshape(-1)  # j = s*128 + p
        wrap = flat.reshape(-1, 16).T  # [16, 8*S_total]
        idxs = np.tile(wrap, (8, 1)).astype(np.int16)  # [128, 8*S_total]
        # trank tensors
        trank = np.ascontiguousarray(trank_slices.T).astype(bf16)
        trankF = trank_slices.reshape(1, -1).astype(bf16)
        # xTcore
        xtc = np.zeros((IN_F, TGT_PAD), np.float32)
        xtc[:, :TGT_PER_CORE] = x[c * TGT_PER_CORE:(c + 1) * TGT_PER_CORE].T
        per_core.append(dict(idxs=idxs, trank=trank, trankF=trankF,
                             xTcore=xtc.astype(bf16)))

    common = dict(xT=xT16, pack0=pack0, pack2=pack2, iotaF=iotaF,
                  iotaP=iotaP, ones1=ones1, W=W, goff=goff,
                  d_tot=d_tot, D_total=D_total, C_total=C_total,
                  S_total=S_total)
    return common, per_core


def cdiv_arr(a, b):
    return -(-a // b)


def _build_program(common):
    W = common["W"]
    d_tot = common["d_tot"]
    goff = common["goff"]
    D_total = common["D_total"]
    C_total = common["C_total"]
    S_total = common["S_total"]

    nc = bacc.Bacc("TRN2", debug=False, num_devices=N_CORES,
                   num_swdge_queues=4)
    f32 = mybir.dt.float32
    bf = mybir.dt.bfloat16
    i16 = mybir.dt.int16

    xT = nc.dram_tensor("xT", [IN_F, N_NODES], bf, kind="ExternalInput").ap()
    xTcore_d = nc.dram_tensor("xTcore", [IN_F, TGT_PAD], bf,
                              kind="ExternalInput").ap()
    pack0_d = nc.dram_tensor("pack0", [IN_F, HF + H], bf,
                             kind="ExternalInput").ap()
    pack2_d = nc.dram_tensor("pack2", [IN_F, HF + H], bf,
                             kind="ExternalInput").ap()
    iotaF_d = nc.dram_tensor("iotaF", [128, 128], bf,
                             kind="ExternalInput").ap()
    iotaP_d = nc.dram_tensor("iotaP", [128, 1], bf,
                             kind="ExternalInput").ap()
    ones1_d = nc.dram_tensor("ones1", [1, 128], bf,
                             kind="ExternalInput").ap()
    idxs_d = nc.dram_tensor("idxs", [128, C_total], i16,
                            kind="ExternalInput").ap()
    trank_d = nc.dram_tensor("trank", [128, D_total], bf,
                             kind="ExternalInput").ap()
    trankF_d = nc.dram_tensor("trankF", [1, 128 * D_total], bf,
                              kind="ExternalInput").ap()
    out_d = nc.dram_tensor("out", [TGT_PAD, HF], f32,
                           kind="ExternalOutput").ap()
    table = nc.dram_tensor("table", [TABLE_ROWS, ROW_ELEMS], bf).ap()

    # block-local slice offsets per chunk
    loc_off = np.zeros((N_BLOCKS, N_CHUNKS), np.int64)
    for b in range(N_BLOCKS):
        loc_off[b] = np.concatenate(([0], np.cumsum(W[b])[:-1]))

    # trank column offsets per block
    tr_off = np.concatenate(([0], np.cumsum(d_tot)[:-1])).astype(np.int64)

    with tile.TileContext(nc) as tc, ExitStack() as ctx:
        consts = ctx.enter_context(tc.tile_pool(name="consts", bufs=1))
        stgB = ctx.enter_context(tc.tile_pool(name="stgB", bufs=4))
        gpool = ctx.enter_context(tc.tile_pool(name="gpool", bufs=2))
        ipool = ctx.enter_context(tc.tile_pool(name="ipool", bufs=2))
        work = ctx.enter_context(tc.tile_pool(name="work", bufs=4))
        workG = ctx.enter_context(tc.tile_pool(name="workG", bufs=2))
        opool = ctx.enter_context(tc.tile_pool(name="opool", bufs=2))
        psA = ctx.enter_context(tc.tile_pool(name="psA", bufs=2,
                                             space="PSUM"))
        psB = ctx.enter_context(tc.tile_pool(name="psB", bufs=2,
                                             space="PSUM"))
        psC = ctx.enter_context(tc.tile_pool(name="psC", bufs=2,
                                             space="PSUM"))
        psD = ctx.enter_context(tc.tile_pool(name="psD", bufs=2,
                                             space="PSUM"))

        from concourse.library_config import mlp
        nc.gpsimd.load_library(mlp)

        # --- constants ------------------------------------------------
        pack0_t = consts.tile([IN_F, HF + H], bf)
        nc.sync.dma_start(out=pack0_t[:], in_=pack0_d[:])
        pack2_t = consts.tile([IN_F, HF + H], bf)
        nc.sync.dma_start(out=pack2_t[:], in_=pack2_d[:])
        iotaF_t = consts.tile([128, 128], bf)
        nc.sync.dma_start(out=iotaF_t[:], in_=iotaF_d[:])
        iotaP_t = consts.tile([128, 1], bf)
        nc.sync.dma_start(out=iotaP_t[:], in_=iotaP_d[:])
        ones1_t = consts.tile([1, 128], bf)
        nc.sync.dma_start(out=ones1_t[:], in_=ones1_d[:])

        # --- Phase B: build table (groups of 4 tiles) ----------------
        for ch in range(N_CHUNKS):
            ntile = cdiv(CHUNK_NODES[ch], 128)
            for t0 in range(0, ntile, 8):
                gt = min(8, ntile - t0)
                n0 = ch * CHUNK + t0 * 128
                gn = min(gt * 128, CHUNK_NODES[ch] - t0 * 128)
                xt = stgB.tile([IN_F, gt * 128], bf, tag="xt")
                nc.sync.dma_start(out=xt[:, :gn], in_=xT[:, n0:n0 + gn])
                rowstg = stgB.tile([128, gt, ROW_ELEMS], bf, tag="row")
                for j in range(gt):
                    nr = min(128, gn - j * 128)
                    pool_j = (psA, psB, psC, psD)[j % 4]
                    tag_j = ("acc", "tb", "bps", "sk")[j % 4]
                    pa = pool_j.tile([128, HF + H], f32, space="PSUM",
                                     tag=tag_j)
                    nc.tensor.matmul(out=pa[:nr], lhsT=xt[:, j * 128:j * 128 + nr],
                                     rhs=pack0_t[:], start=True, stop=True)
                    if j % 2 == 0:
                        nc.vector.tensor_copy(out=rowstg[:nr, j, 0:HF],
                                              in_=pa[:nr, 0:HF])
                    else:
                        nc.scalar.activation(
                            out=rowstg[:nr, j, 0:HF], in_=pa[:nr, 0:HF],
                            func=mybir.ActivationFunctionType.Copy)
                    if j % 2 == 0:
                        nc.scalar.activation(
                            out=rowstg[:nr, j, HF:HF + 2 * H].bitcast(f32),
                            in_=pa[:nr, HF:HF + H],
                            func=mybir.ActivationFunctionType.Copy)
                    else:
                        nc.vector.tensor_copy(
                            out=rowstg[:nr, j, HF:HF + 2 * H].bitcast(f32),
                            in_=pa[:nr, HF:HF + H])
                r0 = CHUNK_BASE[ch] + t0 * 128
                if gn == gt * 128:
                    nc.sync.dma_start(
                        out=table[r0:r0 + gn, :].rearrange(
                            "(j p) c -> p j c", j=gt, p=128),
                        in_=rowstg[:])
                else:
                    for j in range(gt):
                        nr = min(128, gn - j * 128)
                        if nr <= 0:
                            break
                        nc.sync.dma_start(
                            out=table[r0 + j * 128:r0 + j * 128 + nr, :],
                            in_=rowstg[:nr, j, :])

        # --- Phase C ---------------------------------------------------
        call_i = 0
        for g in range(N_GRPS):
            bs = list(range(g * GRP, min((g + 1) * GRP, N_BLOCKS)))
            Sg = int(sum(int(W[b, ch]) for b in bs for ch in range(N_CHUNKS)))
            g0 = goff[(bs[0], 0)]  # first slice of group
            # idx load for the whole group
            idx_t = ipool.tile([128, 8 * Sg], i16, tag="idxg")
            nc.sync.dma_start(out=idx_t[:],
                              in_=idxs_d[:, 8 * g0:8 * (g0 + Sg)])
            # trank loads
            trank_t = ipool.tile([128, Sg], bf, tag="trg")
            nc.sync.dma_start(out=trank_t[:],
                              in_=trank_d[:, g0:g0 + Sg])
            trankF_t = ipool.tile([1, 128 * Sg], bf, tag="trgF")
            nc.sync.dma_start(out=trankF_t[:],
                              in_=trankF_d[:, 128 * g0:128 * (g0 + Sg)])
            # xTcore load
            xpg = ipool.tile([IN_F, len(bs) * 128], bf, tag="xpg")
            nc.sync.dma_start(
                out=xpg[:],
                in_=xTcore_d[:, bs[0] * 128:bs[0] * 128 + len(bs) * 128])

            # gathers per chunk (slices ordered [ch][b in group])
            G = gpool.tile([128, Sg, ROW_ELEMS], bf, tag="G")
            for ch in range(N_CHUNKS):
                wg = int(sum(int(W[b, ch]) for b in bs))
                if wg == 0:
                    continue
                soff = goff[(bs[0], ch)] - g0
                ni = 128 * wg
                tab_ch = table[CHUNK_BASE[ch]:CHUNK_BASE[ch] + CHUNK_ROWS[ch], :]
                nc.gpsimd.dma_gather(
                    G[:, soff:soff + wg, :],
                    tab_ch,
                    idx_t[:, 8 * soff:8 * (soff + wg)],
                    ni, ni, ROW_ELEMS,
                    single_packet=False,
                    queue_num=ch % 4,
                )
                call_i += 1

            ostg = opool.tile([128, len(bs), HF], f32, tag="ostg")
            sk_list = []
            beta_list = []
            for bi, b in enumerate(bs):
                sk_ps = psD.tile([128, HF + H], f32, space="PSUM", tag="sk")
                nc.tensor.matmul(out=sk_ps[:],
                                 lhsT=xpg[:, bi * 128:(bi + 1) * 128],
                                 rhs=pack2_t[:], start=True, stop=True)
                beta_sb = work.tile([128, H], bf, tag="beta")
                nc.scalar.activation(out=beta_sb[:],
                                     in_=sk_ps[:, HF:HF + H],
                                     func=mybir.ActivationFunctionType.Copy)
                sk_sb = work.tile([128, HF], f32, tag="sksb")
                nc.scalar.activation(out=sk_sb[:], in_=sk_ps[:, 0:HF],
                                     func=mybir.ActivationFunctionType.Copy)
                sk_list.append(sk_sb)
                beta_list.append(beta_sb)

            oh_jt = workG.tile([128, Sg, 128], bf, tag="ohjt")
            nc.vector.tensor_tensor(
                out=oh_jt[:],
                in0=trank_t[:].unsqueeze(2).to_broadcast([128, Sg, 128]),
                in1=iotaF_t[:].unsqueeze(1).to_broadcast([128, Sg, 128]),
                op=mybir.AluOpType.is_equal)
            oh_tj = workG.tile([128, Sg, 128], bf, tag="ohtj")
            for q0 in range(0, Sg, 4):
                qn = min(4, Sg - q0)
                tb = psB.tile([128, 512], f32, space="PSUM", tag="tb")
                nc.tensor.matmul(
                    out=tb[:, :qn * 128], lhsT=ones1_t[:],
                    rhs=trankF_t[:, 128 * q0:128 * (q0 + qn)],
                    start=True, stop=True)
                tbc = work.tile([128, 512], bf, tag="tbc")
                nc.scalar.activation(
                    out=tbc[:, :qn * 128], in_=tb[:, :qn * 128],
                    func=mybir.ActivationFunctionType.Copy)
                nc.vector.tensor_tensor(
                    out=oh_tj[:, q0:q0 + qn, :],
                    in0=tbc[:, :qn * 128].rearrange(
                        "t (q j) -> t q j", q=qn),
                    in1=iotaP_t[:].unsqueeze(2).to_broadcast(
                        [128, qn, 128]),
                    op=mybir.AluOpType.is_equal)

            beta_ps = psC.tile([128, Sg * H], f32, space="PSUM", tag="bps")
            gs_lists = []
            for bi, b in enumerate(bs):
                gsl = [goff[(b, ch)] - g0 + k for ch in range(N_CHUNKS)
                       for k in range(int(W[b, ch]))]
                gs_lists.append(gsl)
                for gs in gsl:
                    nc.tensor.matmul(out=beta_ps[:, gs * H:(gs + 1) * H],
                                     lhsT=oh_tj[:, gs, :],
                                     rhs=beta_list[bi][:],
                                     start=True, stop=True)

            s_t = workG.tile([128, Sg, H], f32, tag="st")
            for ch in range(N_CHUNKS):
                wg = int(sum(int(W[b, ch]) for b in bs))
                if wg == 0:
                    continue
                soff = goff[(bs[0], ch)] - g0
                al = G[:, soff:soff + wg, HF:HF + 2 * H].bitcast(f32)
                nc.vector.tensor_tensor(
                    out=s_t[:, soff:soff + wg, :], in0=al,
                    in1=beta_ps[:, soff * H:(soff + wg) * H].rearrange(
                        "p (s h) -> p s h", h=H),
                    op=mybir.AluOpType.add)
            r_t = workG.tile([128, Sg, H], f32, tag="rt")
            nc.scalar.activation(out=r_t[:], in_=s_t[:],
                                 func=mybir.ActivationFunctionType.Relu)
            nc.vector.scalar_tensor_tensor(
                out=s_t[:], in0=r_t[:],
                scalar=(1.0 - NEG_SLOPE) / NEG_SLOPE, in1=s_t[:],
                op0=mybir.AluOpType.mult, op1=mybir.AluOpType.add)
            rhs_t = workG.tile([128, Sg, HF + H], bf, tag="rhs")
            nc.scalar.activation(out=rhs_t[:, :, HF:HF + H],
                                 in_=s_t[:],
                                 func=mybir.ActivationFunctionType.Exp,
                                 scale=NEG_SLOPE)
            for ch in range(N_CHUNKS):
                wg = int(sum(int(W[b, ch]) for b in bs))
                if wg == 0:
                    continue
                soff = goff[(bs[0], ch)] - g0
                nc.vector.tensor_tensor(
                    out=rhs_t[:, soff:soff + wg, 0:HF].rearrange(
                        "p s (h f) -> p s h f", h=H),
                    in0=G[:, soff:soff + wg, 0:HF].rearrange(
                        "p s (h f) -> p s h f", h=H),
                    in1=rhs_t[:, soff:soff + wg, HF:HF + H].unsqueeze(
                        3).to_broadcast([128, wg, H, F]),
                    op=mybir.AluOpType.mult)

            for bi, b in enumerate(bs):
                u_ps = psA.tile([128, HF + H], f32, space="PSUM", tag="acc")
                gsl = gs_lists[bi]
                for i, gs in enumerate(gsl):
                    nc.tensor.matmul(out=u_ps[:], lhsT=oh_jt[:, gs, :],
                                     rhs=rhs_t[:, gs, :],
                                     start=(i == 0),
                                     stop=(i == len(gsl) - 1))
                dinv = work.tile([128, H], f32, tag="dinv")
                nc.vector.tensor_scalar_add(dinv[:], u_ps[:, HF:HF + H], EPS)
                nc.vector.reciprocal(out=dinv[:], in_=dinv[:])
                O = work.tile([128, HF], f32, tag="O")
                nc.vector.tensor_tensor(
                    out=O[:].rearrange("p (h f) -> p h f", h=H),
                    in0=u_ps[:, 0:HF].rearrange("p (h f) -> p h f", h=H),
                    in1=dinv[:].unsqueeze(2).to_broadcast([128, H, F]),
                    op=mybir.AluOpType.mult)
                nc.vector.tensor_tensor(out=O[:], in0=O[:],
                                        in1=sk_list[bi][:],
                                        op=mybir.AluOpType.add)
                R_t = work.tile([128, HF], f32, tag="Rt")
                nc.scalar.activation(out=R_t[:], in_=O[:],
                                     func=mybir.ActivationFunctionType.Relu)
                T_t = work.tile([128, HF], f32, tag="Tt")
                nc.scalar.activation(out=T_t[:], in_=O[:], scale=-1.0,
                                     func=mybir.ActivationFunctionType.Relu)
                E2 = work.tile([128, HF], f32, tag="E2")
                nc.scalar.activation(out=E2[:], in_=T_t[:], scale=-1.0,
                                     func=mybir.ActivationFunctionType.Exp)
                nc.vector.tensor_tensor(out=ostg[:, bi, :], in0=R_t[:],
                                        in1=E2[:], op=mybir.AluOpType.add)
            nc.sync.dma_start(
                out=out_d[bs[0] * 128:bs[0] * 128 + len(bs) * 128, :].rearrange(
                    "(j p) c -> p j c", j=len(bs), p=128),
                in_=ostg[:])

    nc.compile()
    return nc


def kernel(x, edge_index, W_proj, W_skip, a_src, a_tgt):
    common, per_core = _host_prep(x, edge_index, W_proj, W_skip,
                                  a_src, a_tgt)
    key = "prog"
    if key not in _COMPILED:
        _COMPILED[key] = _build_program(common)
    nc = _COMPILED[key]

    in_maps = []
    for c in range(N_CORES):
        pc = per_core[c]
        in_maps.append({
            "xT": common["xT"],
            "xTcore": pc["xTcore"],
            "pack0": common["pack0"],
            "pack2": common["pack2"],
            "iotaF": common["iotaF"],
            "iotaP": common["iotaP"],
            "ones1": common["ones1"],
            "idxs": pc["idxs"],
            "trank": pc["trank"],
            "trankF": pc["trankF"],
        })
    trace = bool(int(os.environ.get("GAT_TRACE", "0")))
    res = run_bass_kernel_spmd(nc, in_maps, list(range(N_CORES)),
                               trace=trace)
    if trace:
        kernel.last_exec_time_ns = res.exec_time_ns
        kernel.last_mean_exec_time_ns = res.mean_exec_time_ns
        kernel.last_trace_path = (res.instructions_and_trace or (None, None))[1]
        kernel.last_profile_json = getattr(res, "profile_json", None)

    out = np.empty((N_NODES, HF), np.float32)
    for c in range(N_CORES):
        o = res.results[c]["out"]  # [12544, 128]
        out[c * TGT_PER_CORE:(c + 1) * TGT_PER_CORE] = o[:TGT_PER_CORE]
    return out


kernel.last_exec_time_ns = None
kernel.last_mean_exec_time_ns = None
kernel.last_trace_path = None
kernel.last_profile_json = None
